# revision 11
# baseline (speedup 1.0000x reference)
"""Cross-attention kernel for TRN2, 8 NeuronCores, data-parallel over batch.

Problem (per full input):
    h_enc: [16, 2048, 1024] f32, h_dec: [512, 16, 1024] f32
    e[b,:,:] = h_enc[b] @ h_dec[:,b,:].T          # [T_enc, T_dec]
    a = softmax(e, axis=T_enc)
    c[b] = a.T @ h_enc[b]                         # [T_dec, D]

Sharding: B=16 -> 2 batches per core (embarrassingly parallel, no
collectives). Each core computes its 2 batches; host concatenates.

v3: all casting happens on-chip (no SWDGE DRAM->DRAM round trip), and
the f32 HBM loads use a (p c) row-to-partition mapping so each SBUF
partition reads CONSECUTIVE DRAM rows -> 8KB contiguous descriptors at
near-peak HBM bandwidth (the (c p) mapping produced 4KB strided
descriptors at ~180 GB/s).

The (p c) mapping permutes the T_enc order seen on-chip; softmax and
the mm2 contraction are invariant to any te permutation as long as
heT (mm1 rhs) and he_nat (mm2 rhs) share it, which they do (both
derive from the same staged tiles). For h_dec the row order defines
the OUTPUT row order, so the host pre-permutes h_dec rows per batch
(hd_dev[4p+c] = hd[c*128+p]); the on-device (p c) load then lands
exactly the identity chunk layout and the output store stays the
plain contiguous pattern.

Per-core plan (fp16 compute on the PE, f32 PSUM accumulation):
  - h_dec halves on the sync queue -> DVE cast -> 4 xbar transposes
  - h_enc 256-row blocks on the scalar queue -> casts alternating
    between DVE and ACT -> 2 xbar transposes each (sync engine)
  - per (batch, t-tile) stage, software-pipelined with its predecessor:
      matmul1: S[128, 2048] += hdT.T @ heT  (8 K-chunks x 4 N-chunks)
      softmax over the free axis: DVE reduce_max(negate) -> ACT
        exp(S+bias) with fused accum_out rowsum -> DVE reciprocal
      P^T on the TensorEngine (16 transpose-mode matmuls via identity),
        packed 4-per-PSUM-bank so DVE does 4 wide copies, not 16
      matmul2: C[128, 1024] += PT.T @ he_nat  (16 K-chunks x 2 N-chunks)
      normalize by 1/rowsum (DVE tensor_scalar_mul), store f32 via the
      scalar-engine HWDGE queue
"""

import os

# Disable the HWDGE sem-increment elision: it assumes FIFO *completion* per
# HW-DGE ring, but 16 SDMA engines drain packets round-robin so a small
# later DMA can land before an earlier big one -> rare consumer race.
os.environ.setdefault("BACC_ELIDE_DMA_OPT_LIMIT", "0")

import numpy as np

import bass_rust
import concourse.bass as bass
import concourse.mybir as mybir
import concourse.tile as tile
from concourse.bass_utils import run_bass_kernel_spmd

FP16 = mybir.dt.float16
F32 = mybir.dt.float32

B_FULL = 16
N_CORES = 8
B_PER_CORE = B_FULL // N_CORES  # 2
T_ENC = 2048
T_DEC = 512
D = 1024
P = 128
E_CHUNKS = T_ENC // P  # 16
D_CHUNKS = D // P      # 8
T_CHUNKS = T_DEC // P  # 4
N1 = 512               # matmul1 N tile (one PSUM bank)
N2 = 512               # matmul2 N tile
WB = 256               # h_enc row-block (T_enc rows) per staging load
N_WB = T_ENC // WB     # 8
WC = WB // P           # te-chunks per block (2)


def split_excess_waits(nc, max_waits: int = 1):
    """This toolchain's walrus accepts only ONE sync-wait command per
    instruction (setupSyncWait raises "Too many sync wait commands"), but
    Tile attaches one wait per producing proc. Hoist excess waits onto
    same-engine NOP carriers inserted just before the instruction."""
    for fn in nc.m.functions:
        for blk in fn.blocks:
            insts = list(blk.instructions)
            new_list = []
            changed = False
            for inst in insts:
                si = inst.sync_info
                waits = list(si.on_wait) if si is not None else []
                if len(waits) > max_waits:
                    changed = True
                    for j, w in enumerate(waits[max_waits:]):
                        nop = mybir.InstNoOp(
                            name=f"{inst.name}-wc{j}",
                            engine=inst.engine,
                            bass_nofuse=True,
                            sync_info=mybir.SyncInfo(on_wait=[w], on_update=[]),
                        )
                        new_list.append(nop)
                    inst.sync_info = bass_rust.SyncInfo(
                        on_wait=waits[:max_waits], on_update=list(si.on_update)
                    )
                new_list.append(inst)
            if changed:
                blk.instructions = new_list


def build_attention_core():
    nc = bass.Bass("TRN2", target_bir_lowering=False, dynamic_dma_scratch_size=4096)
    h_enc = nc.declare_dram_parameter(
        "h_enc", [B_PER_CORE, T_ENC, D], F32, isOutput=False
    )
    # host-pre-permuted: h_dec[b, 4p+c, :] = original hd row c*128+p
    h_dec = nc.declare_dram_parameter(
        "h_dec", [B_PER_CORE, T_DEC, D], F32, isOutput=False
    )
    out = nc.declare_dram_parameter(
        "out", [B_PER_CORE, T_DEC, D], F32, isOutput=True
    )

    with tile.TileContext(nc) as tc:
        with (
            tc.tile_pool(name="stage", bufs=3) as stage_pool,
            tc.tile_pool(name="hd_stage", bufs=1) as hd_stage_pool,
            tc.tile_pool(name="junk", bufs=1) as junk_pool,
            tc.tile_pool(name="hd_nat", bufs=2) as hd_nat_pool,
            tc.tile_pool(name="he_nat", bufs=2) as he_nat_pool,
            tc.tile_pool(name="heT", bufs=2) as heT_pool,
            tc.tile_pool(name="hdT", bufs=2) as hdT_pool,
            tc.tile_pool(name="p", bufs=1) as p_pool,
            tc.tile_pool(name="pt", bufs=1) as pt_pool,
            tc.tile_pool(name="c", bufs=2) as c_pool,
            tc.tile_pool(name="stats", bufs=4) as stats_pool,
            tc.tile_pool(name="psum_s", bufs=1, space="PSUM") as psum_s_pool,
            tc.tile_pool(name="psum_c", bufs=2, space="PSUM") as psum_c_pool,
        ):
            he_nats = {}
            heTs = {}
            hdTs = {}

            junk = junk_pool.tile([P, 2 * N1], FP16)
            nc.gpsimd.memset(junk, 0.0)
            warm_psum = psum_s_pool.tile([P, T_ENC], F32, tag="s_psum")
            for w in range(60):
                nc.tensor.matmul(
                    warm_psum[:, (w % 4) * N1 : (w % 4 + 1) * N1],
                    lhsT=junk[:, 0:P],
                    rhs=junk[:, N1 : 2 * N1],
                    start=True,
                    stop=True,
                )

            def emit_batch_inputs(b):
                # h_dec halves on the sync queue; h_enc blocks on the
                # scalar queue. Each staged tile maps partition p to
                # consecutive DRAM rows (8KB contiguous descriptors).
                hd_nat = hd_nat_pool.tile([P, T_CHUNKS, D], FP16, tag="hd_nat")
                hdT = hdT_pool.tile([P, D_CHUNKS, T_DEC], FP16, tag="hdT")
                he_nat = he_nat_pool.tile([P, E_CHUNKS, D], FP16, tag="he_nat")
                heT = heT_pool.tile([P, D_CHUNKS, T_ENC], FP16, tag="heT")
                hd_src = h_dec.ap()[b].rearrange("(p c) d -> p c d", p=P)
                hd_stages = []
                for h in range(2):
                    hd_stage = hd_stage_pool.tile([P, WC, D], F32, tag="hd_stage")
                    nc.sync.dma_start(
                        out=hd_stage, in_=hd_src[:, 2 * h : 2 * h + 2, :]
                    )
                    hd_stages.append(hd_stage)
                for k in range(N_WB):
                    he_stage = stage_pool.tile([P, WC, D], F32, tag="stage")
                    he_src = h_enc.ap()[b, k * WB : (k + 1) * WB, :].rearrange(
                        "(p c) d -> p c d", p=P
                    )
                    load_eng = nc.gpsimd if k % 2 else nc.scalar
                    load_eng.dma_start(out=he_stage, in_=he_src)
                    # split casts: ACT for even blocks, DVE for odd.
                    # (All-DVE starves SWDGE descriptor gen via the shared
                    # SBUF port pair; all-ACT serializes the cast chain.)
                    if k % 2:
                        nc.vector.tensor_copy(
                            he_nat[:, WC * k : WC * (k + 1), :], he_stage
                        )
                    else:
                        nc.scalar.copy(
                            he_nat[:, WC * k : WC * (k + 1), :], he_stage
                        )
                    for j in range(WC):
                        ec = WC * k + j
                        nc.sync.dma_start(
                            out=heT[:, :, ec * P : (ec + 1) * P],
                            in_=he_nat[:, ec, :],
                            transpose=True,
                        )
                    if k < 2:
                        # interleave the hd chunk this early stage needs
                        h = k
                        nc.scalar.copy(
                            hd_nat[:, 2 * h : 2 * h + 2, :], hd_stages[h]
                        )
                        for tc_i in (2 * h, 2 * h + 1):
                            nc.sync.dma_start(
                                out=hdT[:, :, tc_i * P : (tc_i + 1) * P],
                                in_=hd_nat[:, tc_i, :],
                                transpose=True,
                            )
                he_nats[b] = he_nat
                heTs[b] = heT
                hdTs[b] = hdT

            emit_batch_inputs(0)

            def emit_pt(stage):
                """P^T via ONE xbar DMA transpose [128,2048]->[128,16,128]
                on the sync queue (prior attempts used 16 separate 128-wide
                xbar instrs which serialized badly; a single instr is ~2us)."""
                b, m, p_tile, recip = stage
                pt_tile = pt_pool.tile([P, E_CHUNKS, P], FP16, tag="pt")
                nc.sync.dma_start(out=pt_tile, in_=p_tile, transpose=True)
                return pt_tile

            def emit_mm2(stage, pt_tile):
                b, m, p_tile, recip = stage
                m_sl = slice(m * P, (m + 1) * P)
                he_nat = he_nats[b]
                c_psum = psum_c_pool.tile([P, D], F32, tag="c_psum")
                for ko in range(E_CHUNKS):
                    for no in range(D // N2):
                        nc.tensor.matmul(
                            c_psum[:, no * N2 : (no + 1) * N2],
                            lhsT=pt_tile[:, ko, :],
                            rhs=he_nat[:, ko, no * N2 : (no + 1) * N2],
                            start=(ko == 0),
                            stop=(ko == E_CHUNKS - 1),
                        )
                c_sbuf = c_pool.tile([P, D], F32, tag="c")
                nc.vector.tensor_scalar_mul(c_sbuf, c_psum, recip)
                nc.gpsimd.dma_start(out=out.ap()[b, m_sl, :], in_=c_sbuf)

            prev = None
            for b in range(B_PER_CORE):
                for m in range(T_CHUNKS):
                    heT = heTs[b]
                    hdT = hdTs[b]
                    m_sl = slice(m * P, (m + 1) * P)

                    # ---- matmul1: S = h_dec_tile @ h_enc.T ----
                    s_psum = psum_s_pool.tile([P, T_ENC], F32, tag="s_psum")
                    for no in range(T_ENC // N1):
                        for ko in range(D_CHUNKS):
                            nc.tensor.matmul(
                                s_psum[:, no * N1 : (no + 1) * N1],
                                lhsT=hdT[:, ko, m_sl],
                                rhs=heT[:, ko, no * N1 : (no + 1) * N1],
                                start=(ko == 0),
                                stop=(ko == D_CHUNKS - 1),
                            )

                    # ---- softmax over free axis (T_enc) ----
                    pmax = stats_pool.tile([P, 4], F32, tag="pmax")
                    for no in range(4):
                        nc.vector.tensor_reduce(
                            out=pmax[:, no : no + 1],
                            in_=s_psum[:, no * N1 : (no + 1) * N1],
                            axis=mybir.AxisListType.X,
                            op=mybir.AluOpType.max,
                        )
                    negmax = stats_pool.tile([P, 1], F32, tag="negmax")
                    nc.vector.tensor_reduce(
                        out=negmax,
                        in_=pmax,
                        axis=mybir.AxisListType.X,
                        op=mybir.AluOpType.max,
                        negate=True,
                    )
                    p_tile = p_pool.tile([P, T_ENC], FP16, tag="p")
                    rowsum = stats_pool.tile([P, 1], F32, tag="rowsum")
                    nc.scalar.activation(
                        out=p_tile,
                        in_=s_psum,
                        func=mybir.ActivationFunctionType.Exp,
                        bias=negmax,
                        scale=1.0,
                        accum_out=rowsum,
                    )
                    recip = stats_pool.tile([P, 1], F32, tag="recip")
                    nc.vector.reciprocal(recip, rowsum)
                    pt_cur = emit_pt((b, m, p_tile, recip))

                    # ---- finish the previous stage ----
                    if prev is not None:
                        emit_mm2(*prev)
                    prev = ((b, m, p_tile, recip), pt_cur)

                    # prefetch batch 1 AFTER stage (0,1)'s store is queued
                    # so the store isn't stuck behind 8MB of loads
                    if b == 0 and m == 1:
                        emit_batch_inputs(1)

            emit_mm2(*prev)

    split_excess_waits(nc)
    return nc


_NC_CACHE = None


def _get_nc():
    global _NC_CACHE
    if _NC_CACHE is None:
        _NC_CACHE = build_attention_core()
    return _NC_CACHE


def build_in_maps(h_enc, h_dec):
    """Shard full inputs into per-core in_maps (h_dec rows pre-permuted:
    dev row 4p+c of a batch = original row c*128+p)."""
    in_maps = []
    for i in range(N_CORES):
        sl = slice(i * B_PER_CORE, (i + 1) * B_PER_CORE)
        hd = h_dec[:, sl, :]  # [512, 2, 1024]
        hd_dev = np.ascontiguousarray(
            hd.transpose(1, 0, 2)
            .reshape(B_PER_CORE, T_CHUNKS, P, D)
            .transpose(0, 2, 1, 3)
            .reshape(B_PER_CORE, T_DEC, D)
        )
        in_maps.append(
            {
                "h_enc": np.ascontiguousarray(h_enc[sl]),
                "h_dec": hd_dev,
            }
        )
    return in_maps


def kernel(**inputs) -> np.ndarray:
    h_enc = np.ascontiguousarray(np.asarray(inputs["h_enc"], dtype=np.float32))
    h_dec = np.ascontiguousarray(np.asarray(inputs["h_dec"], dtype=np.float32))
    assert h_enc.shape == (B_FULL, T_ENC, D)
    assert h_dec.shape == (T_DEC, B_FULL, D)

    nc = _get_nc()
    in_maps = build_in_maps(h_enc, h_dec)
    res = run_bass_kernel_spmd(nc, in_maps, core_ids=list(range(N_CORES)))
    out = np.concatenate([res.results[i]["out"] for i in range(N_CORES)], axis=0)
    return np.ascontiguousarray(out.astype(np.float32))


# revision 12
# speedup vs baseline: 1.1702x; 1.1702x over previous
"""Cross-attention kernel for TRN2, 8 NeuronCores, data-parallel over batch.

Problem (per full input):
    h_enc: [16, 2048, 1024] f32, h_dec: [512, 16, 1024] f32
    e[b,:,:] = h_enc[b] @ h_dec[:,b,:].T          # [T_enc, T_dec]
    a = softmax(e, axis=T_enc)
    c[b] = a.T @ h_enc[b]                         # [T_dec, D]

Sharding: B=16 -> 2 batches per core (embarrassingly parallel, no
collectives). Each core computes its 2 batches; host concatenates.

v8: the kernel computes in fp16 on the PE regardless (f32 PSUM accum,
rel_l2 ~1.7e-3), so the host marshaling step casts the shards to fp16
when building the per-core in_maps. That removes the on-device
f32->fp16 staging/cast pipeline whose SBUF staging-buffer WAR convoys,
cast-engine serialization, and DVE-2port-vs-SWDGE port-lock stalls kept
the PE input-starved in v2-v7, and halves DRAM load traffic.

Layouts: h_enc quarters load with a (p c) row-to-partition mapping
(partition p takes 4 consecutive rows -> 8KB contiguous descriptors).
This permutes the on-chip T_enc order; softmax and the mm2 contraction
are invariant to any te permutation as long as heT (mm1 rhs) and he_nat
(mm2 rhs) share it, which they do. For h_dec the row order defines the
OUTPUT row order, so the host pre-permutes rows per batch
(hd_dev[4p+c] = hd[c*128+p]); the (p c) load then lands the identity
chunk layout and the store stays the plain contiguous pattern.

Queues: sync ring carries ONLY xbar transposes (ring FIFO would
otherwise serialize them behind bulk loads); loads split across the
scalar HWDGE ring and the gpsimd SWDGE ring so they run concurrently
(~270 GB/s aggregate measured vs ~160 per ring); stores ride scalar.

Per-core stage pipeline (fp16 PE, f32 PSUM):
    matmul1: S[128, 2048] += hdT.T @ heT  (8 K-chunks x 4 N-chunks)
    softmax over free axis: DVE chunked reduce_max(negate) -> ACT
      exp(S+bias) with fused accum_out rowsum -> DVE reciprocal
    P^T: ONE xbar DMA transpose [128,2048]->[128,16,128] (sync ring)
    matmul2: C[128, 1024] += PT.T @ he_nat  (16 K-chunks x 2 N-chunks)
    scale by 1/rowsum (DVE tensor_scalar_mul, PSUM->SBUF), store f32
~60 junk warmup matmuls run during the load ramp so HAM reaches K=8/8
before the first real mm1 (PSUM junk is cleared by mm1's start=True).
"""

import os

# Disable the HWDGE sem-increment elision: it assumes FIFO *completion* per
# HW-DGE ring, but 16 SDMA engines drain packets round-robin so a small
# later DMA can land before an earlier big one -> rare consumer race
# (nondeterministic NaNs observed before disabling this).
os.environ.setdefault("BACC_ELIDE_DMA_OPT_LIMIT", "0")

import numpy as np

import bass_rust
import concourse.bass as bass
import concourse.mybir as mybir
import concourse.tile as tile
from concourse.bass_utils import run_bass_kernel_spmd

FP16 = mybir.dt.float16
F32 = mybir.dt.float32

B_FULL = 16
N_CORES = 8
B_PER_CORE = B_FULL // N_CORES  # 2
T_ENC = 2048
T_DEC = 512
D = 1024
P = 128
E_CHUNKS = T_ENC // P  # 16
D_CHUNKS = D // P      # 8
T_CHUNKS = T_DEC // P  # 4
N1 = 512               # matmul1 N tile (one PSUM bank)
N2 = 512               # matmul2 N tile
WB = 512               # h_enc rows per load (quarter)
N_WB = T_ENC // WB     # 4
WC = WB // P           # te-chunks per quarter (4)
N_WARM = 60            # HAM warmup matmuls


def split_excess_waits(nc, max_waits: int = 1):
    """This toolchain's walrus accepts only ONE sync-wait command per
    instruction (setupSyncWait raises "Too many sync wait commands"), but
    Tile attaches one wait per producing proc. Hoist excess waits onto
    same-engine NOP carriers inserted just before the instruction."""
    for fn in nc.m.functions:
        for blk in fn.blocks:
            insts = list(blk.instructions)
            new_list = []
            changed = False
            for inst in insts:
                si = inst.sync_info
                waits = list(si.on_wait) if si is not None else []
                if len(waits) > max_waits:
                    changed = True
                    for j, w in enumerate(waits[max_waits:]):
                        nop = mybir.InstNoOp(
                            name=f"{inst.name}-wc{j}",
                            engine=inst.engine,
                            bass_nofuse=True,
                            sync_info=mybir.SyncInfo(on_wait=[w], on_update=[]),
                        )
                        new_list.append(nop)
                    inst.sync_info = bass_rust.SyncInfo(
                        on_wait=waits[:max_waits], on_update=list(si.on_update)
                    )
                new_list.append(inst)
            if changed:
                blk.instructions = new_list


def build_attention_core():
    nc = bass.Bass("TRN2", target_bir_lowering=False, dynamic_dma_scratch_size=4096)
    h_enc = nc.declare_dram_parameter(
        "h_enc", [B_PER_CORE, T_ENC, D], FP16, isOutput=False
    )
    # host-pre-permuted: h_dec[b, 4p+c, :] = original hd row c*128+p
    h_dec = nc.declare_dram_parameter(
        "h_dec", [B_PER_CORE, T_DEC, D], FP16, isOutput=False
    )
    out = nc.declare_dram_parameter(
        "out", [B_PER_CORE, T_DEC, D], F32, isOutput=True
    )

    with tile.TileContext(nc) as tc:
        with (
            tc.tile_pool(name="hd_nat", bufs=2) as hd_nat_pool,
            tc.tile_pool(name="he_nat", bufs=2) as he_nat_pool,
            tc.tile_pool(name="heT", bufs=2) as heT_pool,
            tc.tile_pool(name="hdT", bufs=2) as hdT_pool,
            tc.tile_pool(name="p", bufs=2) as p_pool,
            tc.tile_pool(name="pt", bufs=2) as pt_pool,
            tc.tile_pool(name="c", bufs=2) as c_pool,
            tc.tile_pool(name="stats", bufs=4) as stats_pool,
            tc.tile_pool(name="junk", bufs=1) as junk_pool,
            tc.tile_pool(name="psum_s", bufs=1, space="PSUM") as psum_s_pool,
            tc.tile_pool(name="psum_c", bufs=2, space="PSUM") as psum_c_pool,
        ):
            he_nats = {}
            heTs = {}
            hdTs = {}

            # HAM warmup: junk matmuls into the first s_psum tile keep the
            # PE busy from ~2us so it's at K=8/8 (2.4GHz) when real work
            # arrives; mm1's start=True clears the junk PSUM.
            junk = junk_pool.tile([P, 2 * N1], FP16)
            nc.gpsimd.memset(junk, 0.0)
            warm_psum = psum_s_pool.tile([P, T_ENC], F32, tag="s_psum")
            for w in range(N_WARM):
                nc.tensor.matmul(
                    warm_psum[:, (w % 4) * N1 : (w % 4 + 1) * N1],
                    lhsT=junk[:, 0:P],
                    rhs=junk[:, N1 : 2 * N1],
                    start=True,
                    stop=True,
                )

            def emit_batch_inputs(b):
                # loads: split across the scalar HWDGE ring and the gpsimd
                # SWDGE ring; transposes ONLY on the sync ring.
                hd_nat = hd_nat_pool.tile([P, T_CHUNKS, D], FP16, tag="hd_nat")
                hdT = hdT_pool.tile([P, D_CHUNKS, T_DEC], FP16, tag="hdT")
                he_nat = he_nat_pool.tile([P, E_CHUNKS, D], FP16, tag="he_nat")
                heT = heT_pool.tile([P, D_CHUNKS, T_ENC], FP16, tag="heT")

                hd_src = h_dec.ap()[b].rearrange("(p c) d -> p c d", p=P)
                nc.scalar.dma_start(out=hd_nat, in_=hd_src)
                # tc0's transpose first: stage m=0 needs only hdT cols 0:128
                nc.sync.dma_start(
                    out=hdT[:, :, 0:P], in_=hd_nat[:, 0, :], transpose=True
                )
                for q in range(N_WB):
                    he_src = h_enc.ap()[b, q * WB : (q + 1) * WB, :].rearrange(
                        "(p c) d -> p c d", p=P
                    )
                    eng = nc.gpsimd if q % 2 == 0 else nc.scalar
                    eng.dma_start(
                        out=he_nat[:, WC * q : WC * (q + 1), :], in_=he_src
                    )
                    for j in range(WC):
                        ec = WC * q + j
                        nc.sync.dma_start(
                            out=heT[:, :, ec * P : (ec + 1) * P],
                            in_=he_nat[:, ec, :],
                            transpose=True,
                        )
                    if q == 0:
                        for tc_i in range(1, T_CHUNKS):
                            nc.sync.dma_start(
                                out=hdT[:, :, tc_i * P : (tc_i + 1) * P],
                                in_=hd_nat[:, tc_i, :],
                                transpose=True,
                            )
                he_nats[b] = he_nat
                heTs[b] = heT
                hdTs[b] = hdT

            emit_batch_inputs(0)

            def emit_pt(stage):
                """P^T via ONE xbar DMA transpose [128,2048]->[128,16,128]
                on the sync ring (16 separate 128-wide xbar instrs
                serialize badly; a single instr is ~2us)."""
                b, m, p_tile, recip = stage
                pt_tile = pt_pool.tile([P, E_CHUNKS, P], FP16, tag="pt")
                nc.sync.dma_start(out=pt_tile, in_=p_tile, transpose=True)
                return pt_tile

            def emit_mm2(stage, pt_tile):
                b, m, p_tile, recip = stage
                m_sl = slice(m * P, (m + 1) * P)
                he_nat = he_nats[b]
                c_psum = psum_c_pool.tile([P, D], F32, tag="c_psum")
                for ko in range(E_CHUNKS):
                    for no in range(D // N2):
                        nc.tensor.matmul(
                            c_psum[:, no * N2 : (no + 1) * N2],
                            lhsT=pt_tile[:, ko, :],
                            rhs=he_nat[:, ko, no * N2 : (no + 1) * N2],
                            start=(ko == 0),
                            stop=(ko == E_CHUNKS - 1),
                        )
                c_sbuf = c_pool.tile([P, D], F32, tag="c")
                nc.vector.tensor_scalar_mul(c_sbuf, c_psum, recip)
                nc.scalar.dma_start(out=out.ap()[b, m_sl, :], in_=c_sbuf)

            prev = None
            for b in range(B_PER_CORE):
                for m in range(T_CHUNKS):
                    heT = heTs[b]
                    hdT = hdTs[b]
                    m_sl = slice(m * P, (m + 1) * P)

                    # ---- matmul1: S = h_dec_tile @ h_enc.T ----
                    s_psum = psum_s_pool.tile([P, T_ENC], F32, tag="s_psum")
                    for no in range(T_ENC // N1):
                        for ko in range(D_CHUNKS):
                            nc.tensor.matmul(
                                s_psum[:, no * N1 : (no + 1) * N1],
                                lhsT=hdT[:, ko, m_sl],
                                rhs=heT[:, ko, no * N1 : (no + 1) * N1],
                                start=(ko == 0),
                                stop=(ko == D_CHUNKS - 1),
                            )

                    # ---- softmax over free axis (T_enc) ----
                    pmax = stats_pool.tile([P, 4], F32, tag="pmax")
                    for no in range(4):
                        nc.vector.tensor_reduce(
                            out=pmax[:, no : no + 1],
                            in_=s_psum[:, no * N1 : (no + 1) * N1],
                            axis=mybir.AxisListType.X,
                            op=mybir.AluOpType.max,
                        )
                    negmax = stats_pool.tile([P, 1], F32, tag="negmax")
                    nc.vector.tensor_reduce(
                        out=negmax,
                        in_=pmax,
                        axis=mybir.AxisListType.X,
                        op=mybir.AluOpType.max,
                        negate=True,
                    )
                    p_tile = p_pool.tile([P, T_ENC], FP16, tag="p")
                    rowsum = stats_pool.tile([P, 1], F32, tag="rowsum")
                    nc.scalar.activation(
                        out=p_tile,
                        in_=s_psum,
                        func=mybir.ActivationFunctionType.Exp,
                        bias=negmax,
                        scale=1.0,
                        accum_out=rowsum,
                    )
                    recip = stats_pool.tile([P, 1], F32, tag="recip")
                    nc.vector.reciprocal(recip, rowsum)
                    pt_cur = emit_pt((b, m, p_tile, recip))

                    # ---- finish the previous stage ----
                    if prev is not None:
                        emit_mm2(*prev)
                    prev = ((b, m, p_tile, recip), pt_cur)

                    # prefetch batch 1 AFTER stage (0,1)'s store is queued
                    # so the store isn't stuck behind the b1 loads
                    if b == 0 and m == 1:
                        emit_batch_inputs(1)

            emit_mm2(*prev)

    split_excess_waits(nc)
    return nc


_NC_CACHE = None


def _get_nc():
    global _NC_CACHE
    if _NC_CACHE is None:
        _NC_CACHE = build_attention_core()
    return _NC_CACHE


def build_in_maps(h_enc, h_dec):
    """Shard full f32 inputs into per-core fp16 in_maps. The kernel
    computes in fp16 on the PE either way; casting during marshaling just
    moves the rounding off the device. h_dec rows are pre-permuted
    (dev row 4p+c of a batch = original row c*128+p) so the on-device
    (p c) load lands the identity chunk layout."""
    in_maps = []
    for i in range(N_CORES):
        sl = slice(i * B_PER_CORE, (i + 1) * B_PER_CORE)
        hd = h_dec[:, sl, :]  # [512, 2, 1024]
        hd_dev = np.ascontiguousarray(
            hd.transpose(1, 0, 2)
            .reshape(B_PER_CORE, T_CHUNKS, P, D)
            .transpose(0, 2, 1, 3)
            .reshape(B_PER_CORE, T_DEC, D)
            .astype(np.float16)
        )
        in_maps.append(
            {
                "h_enc": np.ascontiguousarray(h_enc[sl].astype(np.float16)),
                "h_dec": hd_dev,
            }
        )
    return in_maps


def kernel(**inputs) -> np.ndarray:
    h_enc = np.ascontiguousarray(np.asarray(inputs["h_enc"], dtype=np.float32))
    h_dec = np.ascontiguousarray(np.asarray(inputs["h_dec"], dtype=np.float32))
    assert h_enc.shape == (B_FULL, T_ENC, D)
    assert h_dec.shape == (T_DEC, B_FULL, D)

    nc = _get_nc()
    in_maps = build_in_maps(h_enc, h_dec)
    res = run_bass_kernel_spmd(nc, in_maps, core_ids=list(range(N_CORES)))
    out = np.concatenate([res.results[i]["out"] for i in range(N_CORES)], axis=0)
    return np.ascontiguousarray(out.astype(np.float32))


# revision 13
# speedup vs baseline: 1.1924x; 1.0190x over previous
"""Cross-attention kernel for TRN2, 8 NeuronCores, data-parallel over batch.

Problem (per full input):
    h_enc: [16, 2048, 1024] f32, h_dec: [512, 16, 1024] f32
    e[b,:,:] = h_enc[b] @ h_dec[:,b,:].T          # [T_enc, T_dec]
    a = softmax(e, axis=T_enc)
    c[b] = a.T @ h_enc[b]                         # [T_dec, D]

Sharding: B=16 -> 2 batches per core (embarrassingly parallel, no
collectives). Each core computes its 2 batches; host concatenates.

v8: the kernel computes in fp16 on the PE regardless (f32 PSUM accum,
rel_l2 ~1.7e-3), so the host marshaling step casts the shards to fp16
when building the per-core in_maps. That removes the on-device
f32->fp16 staging/cast pipeline whose SBUF staging-buffer WAR convoys,
cast-engine serialization, and DVE-2port-vs-SWDGE port-lock stalls kept
the PE input-starved in v2-v7, and halves DRAM load traffic.

Layouts: h_enc quarters load with a (p c) row-to-partition mapping
(partition p takes 4 consecutive rows -> 8KB contiguous descriptors).
This permutes the on-chip T_enc order; softmax and the mm2 contraction
are invariant to any te permutation as long as heT (mm1 rhs) and he_nat
(mm2 rhs) share it, which they do. For h_dec the row order defines the
OUTPUT row order, so the host pre-permutes rows per batch
(hd_dev[4p+c] = hd[c*128+p]); the (p c) load then lands the identity
chunk layout and the store stays the plain contiguous pattern.

Queues: sync ring carries ONLY xbar transposes (ring FIFO would
otherwise serialize them behind bulk loads); loads split across the
scalar HWDGE ring and the gpsimd SWDGE ring so they run concurrently
(~270 GB/s aggregate measured vs ~160 per ring); stores ride scalar.

Per-core stage pipeline (fp16 PE, f32 PSUM):
    matmul1: S[128, 2048] += hdT.T @ heT  (8 K-chunks x 4 N-chunks)
    softmax over free axis: DVE chunked reduce_max(negate) -> ACT
      exp(S+bias) with fused accum_out rowsum -> DVE reciprocal
    P^T: ONE xbar DMA transpose [128,2048]->[128,16,128] (sync ring)
    matmul2: C[128, 1024] += PT.T @ he_nat  (16 K-chunks x 2 N-chunks)
    scale by 1/rowsum (DVE tensor_scalar_mul, PSUM->SBUF), store f32
~60 junk warmup matmuls run during the load ramp so HAM reaches K=8/8
before the first real mm1 (PSUM junk is cleared by mm1's start=True).
"""

import os

# Disable the HWDGE sem-increment elision: it assumes FIFO *completion* per
# HW-DGE ring, but 16 SDMA engines drain packets round-robin so a small
# later DMA can land before an earlier big one -> rare consumer race
# (nondeterministic NaNs observed before disabling this).
os.environ.setdefault("BACC_ELIDE_DMA_OPT_LIMIT", "0")

import numpy as np

import bass_rust
import concourse.bass as bass
import concourse.mybir as mybir
import concourse.tile as tile
from concourse.bass_utils import run_bass_kernel_spmd

FP16 = mybir.dt.float16
F32 = mybir.dt.float32

B_FULL = 16
N_CORES = 8
B_PER_CORE = B_FULL // N_CORES  # 2
T_ENC = 2048
T_DEC = 512
D = 1024
P = 128
E_CHUNKS = T_ENC // P  # 16
D_CHUNKS = D // P      # 8
T_CHUNKS = T_DEC // P  # 4
N1 = 512               # matmul1 N tile (one PSUM bank)
N2 = 512               # matmul2 N tile
WB = 512               # h_enc rows per load (quarter)
N_WB = T_ENC // WB     # 4
WC = WB // P           # te-chunks per quarter (4)
N_WARM = 60            # HAM warmup matmuls


def split_excess_waits(nc, max_waits: int = 1):
    """This toolchain's walrus accepts only ONE sync-wait command per
    instruction (setupSyncWait raises "Too many sync wait commands"), but
    Tile attaches one wait per producing proc. Hoist excess waits onto
    same-engine NOP carriers inserted just before the instruction."""
    for fn in nc.m.functions:
        for blk in fn.blocks:
            insts = list(blk.instructions)
            new_list = []
            changed = False
            for inst in insts:
                si = inst.sync_info
                waits = list(si.on_wait) if si is not None else []
                if len(waits) > max_waits:
                    changed = True
                    for j, w in enumerate(waits[max_waits:]):
                        nop = mybir.InstNoOp(
                            name=f"{inst.name}-wc{j}",
                            engine=inst.engine,
                            bass_nofuse=True,
                            sync_info=mybir.SyncInfo(on_wait=[w], on_update=[]),
                        )
                        new_list.append(nop)
                    inst.sync_info = bass_rust.SyncInfo(
                        on_wait=waits[:max_waits], on_update=list(si.on_update)
                    )
                new_list.append(inst)
            if changed:
                blk.instructions = new_list


def build_attention_core():
    nc = bass.Bass("TRN2", target_bir_lowering=False, dynamic_dma_scratch_size=4096)
    h_enc = nc.declare_dram_parameter(
        "h_enc", [B_PER_CORE, T_ENC, D], FP16, isOutput=False
    )
    # host-pre-permuted: h_dec[b, 4p+c, :] = original hd row c*128+p
    h_dec = nc.declare_dram_parameter(
        "h_dec", [B_PER_CORE, T_DEC, D], FP16, isOutput=False
    )
    out = nc.declare_dram_parameter(
        "out", [B_PER_CORE, T_DEC, D], F32, isOutput=True
    )

    with tile.TileContext(nc) as tc:
        with (
            tc.tile_pool(name="he_nat", bufs=2) as he_nat_pool,
            tc.tile_pool(name="heT", bufs=2) as heT_pool,
            tc.tile_pool(name="hdT", bufs=2) as hdT_pool,
            tc.tile_pool(name="p", bufs=2) as p_pool,
            tc.tile_pool(name="pt", bufs=2) as pt_pool,
            tc.tile_pool(name="c", bufs=2) as c_pool,
            tc.tile_pool(name="stats", bufs=4) as stats_pool,
            tc.tile_pool(name="junk", bufs=1) as junk_pool,
            tc.tile_pool(name="psum_s", bufs=1, space="PSUM") as psum_s_pool,
            tc.tile_pool(name="psum_c", bufs=2, space="PSUM") as psum_c_pool,
        ):
            he_nats = {}
            heTs = {}
            hdTs = {}

            # HAM warmup: junk matmuls into the first s_psum tile keep the
            # PE busy from ~2us so it's at K=8/8 (2.4GHz) when real work
            # arrives; mm1's start=True clears the junk PSUM.
            junk = junk_pool.tile([P, 2 * N1], FP16)
            nc.gpsimd.memset(junk, 0.0)
            warm_psum = psum_s_pool.tile([P, T_ENC], F32, tag="s_psum")
            for w in range(N_WARM):
                nc.tensor.matmul(
                    warm_psum[:, (w % 4) * N1 : (w % 4 + 1) * N1],
                    lhsT=junk[:, 0:P],
                    rhs=junk[:, N1 : 2 * N1],
                    start=True,
                    stop=True,
                )

            def emit_batch_inputs(b):
                # Transposed operands come straight from DRAM through the
                # xbar (inputs are fp16 now): one [128, 4096] transpose per
                # h_enc quarter -> heT[128, (c dc)=32, 128] per quarter,
                # and one for all of h_dec. No dependency on the natural-
                # layout loads, so the sync ring streams transposes from
                # t~4us. he_nat (mm2 rhs) loads ride scalar+gpsimd rings.
                # h_dec natural layout isn't needed at all.
                hdT = hdT_pool.tile([P, 4 * D_CHUNKS, P], FP16, tag="hdT")
                he_nat = he_nat_pool.tile([P, E_CHUNKS, D], FP16, tag="he_nat")
                heT = heT_pool.tile(
                    [P, N_WB, WC * D_CHUNKS, P], FP16, tag="heT"
                )

                hd_src = h_dec.ap()[b].rearrange("(p c) d -> p (c d)", p=P)
                nc.sync.dma_start(out=hdT, in_=hd_src, transpose=True)
                for q in range(N_WB):
                    he_q = h_enc.ap()[b, q * WB : (q + 1) * WB, :]
                    nc.sync.dma_start(
                        out=heT[:, q, :, :],
                        in_=he_q.rearrange("(p c) d -> p (c d)", p=P),
                        transpose=True,
                    )
                    eng = nc.gpsimd if q % 2 == 0 else nc.scalar
                    eng.dma_start(
                        out=he_nat[:, WC * q : WC * (q + 1), :],
                        in_=he_q.rearrange("(p c) d -> p c d", p=P),
                    )
                he_nats[b] = he_nat
                heTs[b] = heT
                hdTs[b] = hdT

            emit_batch_inputs(0)

            def emit_pt(stage):
                """P^T via ONE xbar DMA transpose [128,2048]->[128,16,128]
                on the sync ring (16 separate 128-wide xbar instrs
                serialize badly; a single instr is ~2us)."""
                b, m, p_tile, recip = stage
                pt_tile = pt_pool.tile([P, E_CHUNKS, P], FP16, tag="pt")
                nc.sync.dma_start(out=pt_tile, in_=p_tile, transpose=True)
                return pt_tile

            def emit_mm2(stage, pt_tile):
                b, m, p_tile, recip = stage
                m_sl = slice(m * P, (m + 1) * P)
                he_nat = he_nats[b]
                c_psum = psum_c_pool.tile([P, D], F32, tag="c_psum")
                for ko in range(E_CHUNKS):
                    for no in range(D // N2):
                        nc.tensor.matmul(
                            c_psum[:, no * N2 : (no + 1) * N2],
                            lhsT=pt_tile[:, ko, :],
                            rhs=he_nat[:, ko, no * N2 : (no + 1) * N2],
                            start=(ko == 0),
                            stop=(ko == E_CHUNKS - 1),
                        )
                c_sbuf = c_pool.tile([P, D], F32, tag="c")
                nc.vector.tensor_scalar_mul(c_sbuf, c_psum, recip)
                nc.scalar.dma_start(out=out.ap()[b, m_sl, :], in_=c_sbuf)

            prev = None
            for b in range(B_PER_CORE):
                for m in range(T_CHUNKS):
                    heT = heTs[b]
                    hdT = hdTs[b]
                    m_sl = slice(m * P, (m + 1) * P)

                    # ---- matmul1: S = h_dec_tile @ h_enc.T ----
                    s_psum = psum_s_pool.tile([P, T_ENC], F32, tag="s_psum")
                    for no in range(T_ENC // N1):
                        for ko in range(D_CHUNKS):
                            nc.tensor.matmul(
                                s_psum[:, no * N1 : (no + 1) * N1],
                                lhsT=hdT[:, m * D_CHUNKS + ko, :],
                                rhs=heT[:, no, ko :: D_CHUNKS, :],
                                start=(ko == 0),
                                stop=(ko == D_CHUNKS - 1),
                            )

                    # ---- softmax over free axis (T_enc) ----
                    pmax = stats_pool.tile([P, 4], F32, tag="pmax")
                    for no in range(4):
                        nc.vector.tensor_reduce(
                            out=pmax[:, no : no + 1],
                            in_=s_psum[:, no * N1 : (no + 1) * N1],
                            axis=mybir.AxisListType.X,
                            op=mybir.AluOpType.max,
                        )
                    negmax = stats_pool.tile([P, 1], F32, tag="negmax")
                    nc.vector.tensor_reduce(
                        out=negmax,
                        in_=pmax,
                        axis=mybir.AxisListType.X,
                        op=mybir.AluOpType.max,
                        negate=True,
                    )
                    p_tile = p_pool.tile([P, T_ENC], FP16, tag="p")
                    rowsum = stats_pool.tile([P, 1], F32, tag="rowsum")
                    nc.scalar.activation(
                        out=p_tile,
                        in_=s_psum,
                        func=mybir.ActivationFunctionType.Exp,
                        bias=negmax,
                        scale=1.0,
                        accum_out=rowsum,
                    )
                    recip = stats_pool.tile([P, 1], F32, tag="recip")
                    nc.vector.reciprocal(recip, rowsum)
                    pt_cur = emit_pt((b, m, p_tile, recip))

                    # ---- finish the previous stage ----
                    if prev is not None:
                        emit_mm2(*prev)
                    prev = ((b, m, p_tile, recip), pt_cur)

                    # prefetch batch 1 AFTER stage (0,1)'s store is queued
                    # so the store isn't stuck behind the b1 loads
                    if b == 0 and m == 1:
                        emit_batch_inputs(1)

            emit_mm2(*prev)

    split_excess_waits(nc)
    return nc


_NC_CACHE = None


def _get_nc():
    global _NC_CACHE
    if _NC_CACHE is None:
        _NC_CACHE = build_attention_core()
    return _NC_CACHE


def build_in_maps(h_enc, h_dec):
    """Shard full f32 inputs into per-core fp16 in_maps. The kernel
    computes in fp16 on the PE either way; casting during marshaling just
    moves the rounding off the device. h_dec rows are pre-permuted
    (dev row 4p+c of a batch = original row c*128+p) so the on-device
    (p c) load lands the identity chunk layout."""
    in_maps = []
    for i in range(N_CORES):
        sl = slice(i * B_PER_CORE, (i + 1) * B_PER_CORE)
        hd = h_dec[:, sl, :]  # [512, 2, 1024]
        hd_dev = np.ascontiguousarray(
            hd.transpose(1, 0, 2)
            .reshape(B_PER_CORE, T_CHUNKS, P, D)
            .transpose(0, 2, 1, 3)
            .reshape(B_PER_CORE, T_DEC, D)
            .astype(np.float16)
        )
        in_maps.append(
            {
                "h_enc": np.ascontiguousarray(h_enc[sl].astype(np.float16)),
                "h_dec": hd_dev,
            }
        )
    return in_maps


def kernel(**inputs) -> np.ndarray:
    h_enc = np.ascontiguousarray(np.asarray(inputs["h_enc"], dtype=np.float32))
    h_dec = np.ascontiguousarray(np.asarray(inputs["h_dec"], dtype=np.float32))
    assert h_enc.shape == (B_FULL, T_ENC, D)
    assert h_dec.shape == (T_DEC, B_FULL, D)

    nc = _get_nc()
    in_maps = build_in_maps(h_enc, h_dec)
    res = run_bass_kernel_spmd(nc, in_maps, core_ids=list(range(N_CORES)))
    out = np.concatenate([res.results[i]["out"] for i in range(N_CORES)], axis=0)
    return np.ascontiguousarray(out.astype(np.float32))
